# revision 9
# baseline (speedup 1.0000x reference)
"""Separable 25-tap Gaussian blur (sigma=4, KRAD=12) on [1,3,4096,4096] fp32.

Strategy (8 NeuronCores, shard H into 8 strips of 512 rows + 12-row halos):
  - Host: per-core slice with edge-replicated halos, pad, transpose to
    [C, W_padded, H_padded], and cast to bf16 (halves input DMA; blur output
    error from bf16 staging is ~2e-3 relative, far under tolerance).
  - Pass A (horizontal conv): banded-stationary bf16 matmuls on TensorE.
      out[w_out, h] = sum_d g[d] * x[w_out + d, h]; band matrix stationary,
      contraction over w on partitions.  Two matmuls per 128-row output tile
      (K=128 aligned + K=32 halo from the next tile), accumulated in PSUM.
  - Middle transpose: TensorE transpose (128x128 bf16 blocks via identity),
    PSUM -> DVE copy -> SBUF, yielding h-on-partitions layout.
  - Pass B (vertical conv): same banded-stationary structure; output fp32
    row-major [h, w]; ACT copies PSUM->SBUF, DMA to DRAM.

Note: reference divides by img.max() and multiplies back after the blur;
by linearity of convolution this cancels (up to fp rounding far below the
tolerance), so the kernel is a pure separable depthwise convolution.
"""

import numpy as np
import ml_dtypes

import concourse.bass as bass
import concourse.mybir as mybir
import concourse.tile as tile
from concourse import bacc
from concourse.bass_utils import run_bass_kernel_spmd

SIGMA = 4.0
KRAD = 12
TAPS = 2 * KRAD + 1  # 25

C = 3
H = 4096
W = 4096
NCORES = 8
HS = H // NCORES          # 512 output rows per core
HHALO = HS + 2 * KRAD     # 536 input rows per core
HP = 544                  # h padded (multiple of 32, >= HHALO)
WHALO = W + 2 * KRAD      # 4120 input cols
WP = 4128                 # w padded so last tile has 32 partitions
NT = 33                   # input tile columns: 32 x 128 + 1 x 32
RW = 512                  # w region width
NR = W // RW              # 8 regions
HCHUNKS = ((0, 288), (288, 256))  # pass-A moving chunks over HP=544


def gauss_taps() -> np.ndarray:
    x = np.arange(-KRAD, KRAD + 1, dtype=np.float32)
    k = np.exp(-(x * x) / np.float32(2.0 * SIGMA * SIGMA)).astype(np.float32)
    return (k / k.sum()).astype(np.float32)


def band_matrices():
    """A0 [128,128], A1 [32,128]: out[q] = sum_p A0[p,q]*x[p] + A1[p2,q]*x[128+p2]
    where x holds the 152-wide input window [q0, q0+152) and out is 128 wide."""
    g = gauss_taps()
    A0 = np.zeros((128, 128), np.float32)
    A1 = np.zeros((32, 128), np.float32)
    for q in range(128):
        for d in range(TAPS):
            p = q + d
            if p < 128:
                A0[p, q] = g[d]
            else:
                A1[p - 128, q] = g[d]
    return A0, A1


def build_nc(reps: int = 1):
    nc = bacc.Bacc(
        "TRN2", target_bir_lowering=False, debug=False, num_devices=NCORES
    )
    f32 = mybir.dt.float32
    bf16 = mybir.dt.bfloat16

    xT = nc.dram_tensor("xT", [C, WP, HP], bf16, kind="ExternalInput")
    A0d = nc.dram_tensor("A0", [128, 128], bf16, kind="ExternalInput")
    A1d = nc.dram_tensor("A1", [32, 128], bf16, kind="ExternalInput")
    IDd = nc.dram_tensor("ID", [128, 128], bf16, kind="ExternalInput")
    y = nc.dram_tensor("y", [C, HS, W], f32, kind="ExternalOutput")

    with tile.TileContext(nc) as tc:
        with (
            tc.tile_pool(name="const", bufs=1) as const_pool,
            tc.tile_pool(name="inp", bufs=14) as in_pool,
            tc.tile_pool(name="ya", bufs=6) as ya_pool,
            tc.tile_pool(name="z", bufs=12) as z_pool,
            tc.tile_pool(name="ob", bufs=6) as out_pool,
            tc.tile_pool(name="psA", bufs=4, space=bass.MemorySpace.PSUM) as psA_pool,
            tc.tile_pool(name="psT", bufs=2, space=bass.MemorySpace.PSUM) as psT_pool,
            tc.tile_pool(name="psB", bufs=2, space=bass.MemorySpace.PSUM) as psB_pool,
        ):
            cA0 = const_pool.tile([128, 128], bf16, name="cA0")
            nc.sync.dma_start(out=cA0[:], in_=A0d[:])
            cA1 = const_pool.tile([32, 128], bf16, name="cA1")
            nc.sync.dma_start(out=cA1[:], in_=A1d[:])
            cID = const_pool.tile([128, 128], bf16, name="cID")
            nc.sync.dma_start(out=cID[:], in_=IDd[:])

            for rep in range(reps):
              for c in range(C):
                tiles = {}

                def get_tile(t, c=c, tiles=tiles):
                    if t not in tiles:
                        pw = 128 if t < NT - 1 else 32
                        it = in_pool.tile([pw, HP], bf16, tag="inp", name=f"in{c}_{t}")
                        nc.sync.dma_start(
                            out=it[:], in_=xT[c, 128 * t : 128 * t + pw, :]
                        )
                        tiles[t] = it
                    return tiles[t]

                for R in range(NR):
                    # ---- pass A: horizontal conv, out [w_out 128, h] ----
                    ya = []
                    for s in range(4):
                        t = 4 * R + s
                        it0 = get_tile(t)
                        it1 = get_tile(t + 1)
                        yat = ya_pool.tile(
                            [128, HP], bf16, tag="ya", name=f"ya{c}_{R}_{s}"
                        )
                        for h0, hn in HCHUNKS:
                            ps = psA_pool.tile([128, 288], f32, tag="psA", name="psA")
                            nc.tensor.matmul(
                                ps[:, 0:hn],
                                cA0[:],
                                it0[:, h0 : h0 + hn],
                                start=True,
                                stop=False,
                            )
                            nc.tensor.matmul(
                                ps[:, 0:hn],
                                cA1[:],
                                it1[0:32, h0 : h0 + hn],
                                start=False,
                                stop=True,
                            )
                            nc.vector.tensor_copy(yat[:, h0 : h0 + hn], ps[:, 0:hn])
                        ya.append(yat)

                    # ---- transpose to [h, w-within-region] via TensorE ----
                    zs = []
                    for hb in range(5):
                        hn = 128 if hb < 4 else 32
                        pt = psT_pool.tile([128, 512], bf16, tag="psT", name="psT")
                        for s in range(4):
                            nc.tensor.transpose(
                                pt[0:hn, 128 * s : 128 * s + 128],
                                ya[s][:, 128 * hb : 128 * hb + hn],
                                cID[:],
                            )
                        zt = z_pool.tile([128, 512], bf16, tag="z", name=f"z{hb}")
                        # balance PSUM->SBUF copy load across DVE and ACT
                        if hb % 5 < 2:
                            nc.vector.tensor_copy(zt[0:hn, :], pt[0:hn, :])
                        else:
                            nc.scalar.copy(zt[0:hn, :], pt[0:hn, :])
                        zs.append(zt)

                    # ---- pass B: vertical conv, out [h_out 128, w 512] ----
                    for q in range(4):
                        pb = psB_pool.tile([128, 512], f32, tag="psB", name="psB")
                        nc.tensor.matmul(
                            pb[:],
                            cA0[:],
                            zs[q][:, :],
                            start=True,
                            stop=False,
                        )
                        nc.tensor.matmul(
                            pb[:],
                            cA1[:],
                            zs[q + 1][0:32, :],
                            start=False,
                            stop=True,
                        )
                        ob = out_pool.tile([128, 512], f32, tag="ob", name="ob")
                        nc.scalar.copy(ob[:], pb[:])
                        nc.sync.dma_start(
                            out=y[c, 128 * q : 128 * q + 128, RW * R : RW * R + RW],
                            in_=ob[:],
                        )

    nc.compile()
    return nc


def host_inputs(img: np.ndarray):
    """img [1,C,H,W] fp32 -> per-core input dicts."""
    img0 = np.asarray(img)[0]  # [C, H, W]
    A0, A1 = band_matrices()
    bf = ml_dtypes.bfloat16
    A0 = A0.astype(bf)
    A1 = A1.astype(bf)
    ID = np.eye(128, dtype=np.float32).astype(bf)
    widx = np.clip(np.arange(WP) - KRAD, 0, W - 1)
    in_maps = []
    for i in range(NCORES):
        hidx = np.clip(np.arange(HP) + (HS * i - KRAD), 0, H - 1)
        xc = img0[:, hidx][:, :, widx]  # [C, HP, WP]
        xT = np.ascontiguousarray(xc.transpose(0, 2, 1)).astype(bf)  # [C, WP, HP]
        in_maps.append({"xT": xT, "A0": A0, "A1": A1, "ID": ID})
    return in_maps


_NC_CACHE = {}


def _get_nc(reps: int = 1):
    if reps not in _NC_CACHE:
        _NC_CACHE[reps] = build_nc(reps)
    return _NC_CACHE[reps]


def run(img: np.ndarray, trace: bool = False, reps: int = 1, in_maps=None):
    nc = _get_nc(reps)
    if in_maps is None:
        in_maps = host_inputs(img)
    res = run_bass_kernel_spmd(
        nc, in_maps, core_ids=list(range(NCORES)), trace=trace
    )
    out = np.concatenate([res.results[i]["y"] for i in range(NCORES)], axis=1)
    return out, res


def kernel(img: np.ndarray) -> np.ndarray:
    out, _ = run(img, trace=False)
    return out.astype(np.float32, copy=False)
